# revision 23
# baseline (speedup 1.0000x reference)
"""Trainium2 Bass kernel for BiaffinePairing.

Computes S = (T @ W) @ A^T + T @ U[:H] + (A @ U[H:]).T + b  -> [4096, 4096] f32.

Strategy (8 NeuronCores, data-parallel over T's row dim n):
  - Host-side layout prep only (no math): transpose T and A so the
    contraction dim H=1024 lies on SBUF partitions; shard T^T's columns
    (the n dim) 8 ways; replicate A^T, W, and the U halves. Matmul inputs
    are pre-cast to fp16 on the host (~4e-4 relative error end-to-end).
    All DRAM-side operands are additionally permuted so that every DMA the
    kernel issues reads per-partition-contiguous lines (descriptor size is
    what sets DMA throughput, especially while the SDMA engines are still
    cold in the first ~15 us of the kernel).
  - Per core: mm1 computes TWt[h_out, n] = (T_shard @ W)^T accumulating in
    PSUM; the rank-1 term 1_n (x) (A @ u_a)^T folds in by adding u_a[h] as
    a per-partition bias on mm1's PSUM->SBUF copy (since
    (TW + 1 (x) u_a^T) @ A^T = TW@A^T + 1 (x) (A@u_a)^T).
  - tvec[n] = T_shard @ u_t + b via tiny matmuls; added as the per-partition
    bias on mm2's PSUM->SBUF copies.
  - mm2 computes S_shard[n, m] = sum_k TWt[k]^T @ At[k] over m-chunks.

Schedule notes (why the structure looks the way it is):
  - The PE stream rate is the wall: 320 N=512 matmuls run back-to-back at
    512 cycles each (s2s 216 ns at 2.4 GHz; 259 ns when the package power
    manager drops the cores to 2.0 GHz under sustained 8-core load). The
    optimization surface is the schedule fat around that stream.
  - Load order: W k0 / tT k0 split across both HWDGE FIFOs first (mm1's
    gate), then the remaining k-tiles alternating FIFOs, then the A
    chunks as 8 x 1 MB column-chunk DMAs (8 KB lines, chunk-major SBUF
    layout) alternating FIFOs. ut/ua ride the GpSimd SWDGE queue so they
    cost no HWDGE issue slots.
  - A tapered warmup burst of small matmuls keeps the PE HAM activity
    window busy from right after the framework preamble so the clock gate
    reaches 8/8 while mm1 streams, and bridges until k0 lands.
  - Outputs store as fp16 (upcast on host): halves store bytes and drain.
    Stores alternate FIFOs; the final tile stores in 4 slices so the
    end-of-kernel drain is short.
"""

import numpy as np

import concourse.bacc as bacc
import concourse.mybir as mybir
from concourse.tile import TileContext
from concourse.tile_rust import add_dep_helper
from concourse.bass_utils import run_bass_kernel_spmd

H = 1024          # hidden dim (contraction)
N_TOT = 4096      # rows of target_spans
M_TOT = 4096      # rows of argument_spans
N_CORES = 8
NSH = N_TOT // N_CORES   # 512 n rows per core
KT = H // 128            # 8 contraction k-tiles
NI = NSH // 128          # 4 n-tiles of 128 per core
MCH = 512                # m column-chunk width (one PSUM tile per chunk)
MC = M_TOT // MCH        # 8 m-chunks

F32 = mybir.dt.float32
F16 = mybir.dt.float16

_NC_CACHE = {}


def _build(b_val: float, warm: int = 8):
    nc = bacc.Bacc("TRN2", target_bir_lowering=False, debug=False,
                   num_devices=N_CORES)

    # All inputs are host-permuted so partition p's data is contiguous in
    # DRAM (see make_in_maps): dim0 is the SBUF partition.
    # tT carries 4 extra columns per k-tile: [512:514] = u_t duplicated
    # (the 2-wide tvec moving operand), [514] = u_a as fp16 (bias for
    # mm1's copy-out; u_a ~ N(0, 0.02^2) so fp16 adds ~1e-5 abs error),
    # [515] = pad. Folding them into tT's k-tile DMAs avoids two
    # tiny-descriptor loads that would round-robin against the critical
    # W/tT stream.
    TTW = NSH + 4
    tT = nc.dram_tensor("tT", [128, KT, TTW], F16, kind="ExternalInput")
    aT = nc.dram_tensor("aT", [128, MC, KT, MCH], F16, kind="ExternalInput")
    # W is half-major: [p, half, k, 512] — mm1's pass A only touches
    # half 0, so the bytes racing mm1's k-consumption halve (the full
    # W+tT k-pair rate would need ~440 GB/s, over the 358 HBM ceiling).
    W = nc.dram_tensor("W", [128, 2, KT, 512], F16, kind="ExternalInput")
    out = nc.dram_tensor("out", [NSH, M_TOT], F16, kind="ExternalOutput")

    with TileContext(nc) as tc:
        with (
            tc.tile_pool(name="const", bufs=1) as cpool,
            tc.tile_pool(name="outbuf", bufs=6) as opool,
            tc.tile_pool(name="ps1", bufs=1, space="PSUM") as ps1pool,
            tc.tile_pool(name="ps2", bufs=4, space="PSUM") as ps2pool,
        ):
            w_sb = cpool.tile([128, 2, KT, 512], F16, tag="w")
            tT_sb = cpool.tile([128, KT, TTW], F16, tag="tT")
            at_sb = cpool.tile([128, MC, KT, MCH], F16, tag="at")

            # ---- load DMAs. k0's W/tT go first, each split across both
            # HWDGE FIFOs (halving first-tile latency on the cold SDMA
            # engines); later k-tiles alternate FIFOs; the A column-chunks
            # follow; ut/ua ride the SWDGE queue. ----
            nc.sync.dma_start(out=w_sb[0:64, 0, 0, :], in_=W[0:64, 0, 0, :])
            nc.scalar.dma_start(out=w_sb[64:128, 0, 0, :],
                                in_=W[64:128, 0, 0, :])
            nc.sync.dma_start(out=tT_sb[0:64, 0, :], in_=tT[0:64, 0, :])
            nc.scalar.dma_start(out=tT_sb[64:128, 0, :], in_=tT[64:128, 0, :])
            for k in range(1, KT):
                eng_w = nc.sync if k % 2 == 0 else nc.scalar
                eng_t = nc.scalar if k % 2 == 0 else nc.sync
                eng_w.dma_start(out=w_sb[:, 0, k, :], in_=W[:, 0, k, :])
                eng_t.dma_start(out=tT_sb[:, k, :], in_=tT[:, k, :])
            # pass B's W half streams behind pass A's critical set.
            wh1_last = {}
            for k in range(KT):
                eng = nc.sync if k % 2 == 0 else nc.scalar
                wh1_last[k % 2] = eng.dma_start(out=w_sb[:, 1, k, :],
                                                in_=W[:, 1, k, :])
            for c in range(MC):
                eng = nc.sync if c % 2 == 0 else nc.scalar
                dma = eng.dma_start(out=at_sb[:, c, :, :], in_=aT[:, c, :, :])
                if c < 2:
                    # Gate the first A-chunks on pass B's W tiles: once one
                    # HWDGE ring runs dry, the SDMA round-robin lets these
                    # big chunks crowd out the other ring's W tiles, which
                    # mm1 pass B stalls on (seen as a 1-4 us PE gap).
                    for w_i in wh1_last.values():
                        add_dep_helper(dma.ins, w_i.ins, sync=True,
                                       reason="at-chunks after pass-B W")

            # ---- PE warmup: the ~7.3us framework preamble leaves the PE
            # idle, so the HAM clock-gate sits at K=4/8 (half clock), and
            # mm1's first k-tile only lands ~5 us later (cold SDMA engines
            # move ~1 descriptor/530ns). Dummy matmuls bridge that whole
            # window gaplessly: a few short ones the moment the first
            # memset lands, then N=512 ones (427 ns each cold). The HAM
            # busy-window then trips ~3.4 us in, so mm1 streams at full
            # clock from its first matmul. ----
            warm_w = cpool.tile([128, 128], F16, tag="warm_w")
            warm_in = cpool.tile([128, NSH], F16, tag="warm_in")
            nc.vector.memset(warm_w[:], 0.0)
            nc.vector.memset(warm_in[:], 0.0)
            wps = ps1pool.tile([128, NSH], F32, tag="ps1_0", name="wps")
            for _ in range(3):
                nc.tensor.matmul(wps[:, 0:128], warm_w[:], warm_w[:],
                                 start=True, stop=True)
            for _ in range(warm):
                nc.tensor.matmul(wps[:], warm_w[:], warm_in[:],
                                 start=True, stop=True)

            # u_a arrives packed as fp16 column NSH+2 of tT; the engines
            # want an f32 bias/scalar AP, so expand it once on DVE.
            ua_sb = cpool.tile([128, KT], F32, tag="ua")
            nc.vector.tensor_copy(out=ua_sb[:], in_=tT_sb[:, :, NSH + 2])

            # ---- mm1: TWt[h_out, n] = (T @ W)^T, + u_a bias on copy-out.
            # Two ho-half passes, k-outer over 4 PSUM banks each: a k step
            # only needs the W/tT k-tile k, so the PE chases the loads. ----
            twt_sb = cpool.tile([128, KT, NSH], F16, tag="twt")
            ps1 = [ps1pool.tile([128, NSH], F32, tag=f"ps1_{j}",
                                name=f"ps1_{j}")
                   for j in range(4)]
            for half in range(2):
                for k in range(KT):
                    for j in range(4):
                        ho = half * 4 + j
                        nc.tensor.matmul(
                            ps1[j][:],
                            w_sb[:, half, k, j * 128:(j + 1) * 128],
                            tT_sb[:, k, 0:NSH],
                            start=(k == 0),
                            stop=(k == KT - 1),
                        )
                for j in range(4):
                    ho = half * 4 + j
                    # TWt[ho] = psum + u_a[ho-tile] (per-partition bias),
                    # cast to fp16 for mm2; alternate DVE/ACT so the
                    # copies gating mm2's start aren't serialized.
                    if j % 2 == 0:
                        nc.vector.tensor_scalar_add(
                            out=twt_sb[:, ho, :], in0=ps1[j][:],
                            scalar1=ua_sb[:, ho:ho + 1],
                        )
                    else:
                        nc.scalar.activation(
                            out=twt_sb[:, ho, :], in_=ps1[j][:],
                            func=mybir.ActivationFunctionType.Identity,
                            bias=ua_sb[:, ho:ho + 1],
                        )

            # ---- tvec[n] = T @ u_t + b: 32 tiny matmuls (ut is the 2-wide
            # moving operand; psum column 0 is tvec). ----
            tvec_sb = cpool.tile([128, NI], F32, tag="tvec")
            for ni in range(NI):
                psv = ps2pool.tile([128, 2], F32, tag="ps", name="psv")
                for k in range(KT):
                    nc.tensor.matmul(
                        psv[:],
                        tT_sb[:, k, ni * 128:(ni + 1) * 128],
                        tT_sb[:, k, NSH:NSH + 2],
                        start=(k == 0),
                        stop=(k == KT - 1),
                    )
                nc.scalar.activation(
                    out=tvec_sb[:, ni:ni + 1], in_=psv[:, 0:1],
                    func=mybir.ActivationFunctionType.Identity,
                    bias=float(b_val),
                )

            # ---- mm2: S[n, m] = sum_k TWt[k]^T @ At[k], + tvec bias.
            # Output tiles store as fp16; copies alternate DVE/ACT and
            # store dma_starts alternate FIFOs. The very last tile is
            # copied and stored in 4 column slices across both FIFOs so
            # the end-of-kernel drain after the final matmul is short. ----
            tile_idx = 0
            n_tiles = MC * NI
            for c in range(MC):
                for ni in range(NI):
                    ps = ps2pool.tile([128, 512], F32, tag="ps", name="ps")
                    for k in range(KT):
                        nc.tensor.matmul(
                            ps[:],
                            twt_sb[:, k, ni * 128:(ni + 1) * 128],
                            at_sb[:, c, k, :],
                            start=(k == 0),
                            stop=(k == KT - 1),
                        )
                    last = tile_idx == n_tiles - 1
                    o_sb = opool.tile([128, 512], F16, tag="o")
                    if not last:
                        if tile_idx % 2 == 0:
                            nc.vector.tensor_scalar_add(
                                out=o_sb[:], in0=ps[:],
                                scalar1=tvec_sb[:, ni:ni + 1],
                            )
                            st_eng = nc.scalar
                        else:
                            nc.scalar.activation(
                                out=o_sb[:], in_=ps[:],
                                func=mybir.ActivationFunctionType.Identity,
                                bias=tvec_sb[:, ni:ni + 1],
                            )
                            st_eng = nc.sync
                        st_eng.dma_start(
                            out=out[ni * 128:(ni + 1) * 128,
                                    c * MCH:(c + 1) * MCH],
                            in_=o_sb[:],
                        )
                    else:
                        # Emit the ACT copy first so it isn't queued
                        # behind a store-issue on the scalar engine; the
                        # two copies then run concurrently on ACT/DVE and
                        # the two stores issue concurrently on sync/scalar.
                        s0, s1 = slice(0, 256), slice(256, 512)
                        nc.scalar.activation(
                            out=o_sb[:, s1], in_=ps[:, s1],
                            func=mybir.ActivationFunctionType.Identity,
                            bias=tvec_sb[:, ni:ni + 1],
                        )
                        nc.vector.tensor_scalar_add(
                            out=o_sb[:, s0], in0=ps[:, s0],
                            scalar1=tvec_sb[:, ni:ni + 1],
                        )
                        nc.sync.dma_start(
                            out=out[ni * 128:(ni + 1) * 128,
                                    c * MCH + 256:(c + 1) * MCH],
                            in_=o_sb[:, s1],
                        )
                        nc.scalar.dma_start(
                            out=out[ni * 128:(ni + 1) * 128,
                                    c * MCH:c * MCH + 256],
                            in_=o_sb[:, s0],
                        )
                    tile_idx += 1

    nc.compile()
    return nc


def _get_nc(b_val: float):
    key = float(b_val)
    if key not in _NC_CACHE:
        _NC_CACHE[key] = _build(key)
    return _NC_CACHE[key]


def make_in_maps(target_spans, argument_spans, W, U, b):
    """Host-side layout prep: shard/transpose/cast/permute the full inputs
    into the per-core input maps. Returns (in_maps, b_val).

    Permutations put SBUF partition p's data contiguous in DRAM:
      W    [128, KT, H]       W_perm[p, k, f]    = W[k*128+p, f]
      tT   [128, KT, NSH]     (per core shard)   = T[n0+n, k*128+p]^T
      aT   [128, MC, KT, MCH] chunk-major        = A[c*MCH+m, k*128+p]^T
    """
    target_spans = np.asarray(target_spans, dtype=np.float32)
    argument_spans = np.asarray(argument_spans, dtype=np.float32)
    W = np.asarray(W, dtype=np.float32)
    U = np.asarray(U, dtype=np.float32).reshape(2 * H, 1)
    b_val = float(np.asarray(b).reshape(-1)[0])

    # [H, X] -> [128, KT, X] with row k*128+p -> [p, k]
    def perm_kp(mat_hx):
        return np.ascontiguousarray(
            mat_hx.reshape(KT, 128, -1).transpose(1, 0, 2))

    # W half-major: [128, 2, KT, 512], W_p[p, h, k, f] = W[k*128+p, h*512+f]
    W_p = np.ascontiguousarray(
        W.astype(np.float16).reshape(KT, 128, 2, 512).transpose(1, 2, 0, 3))
    tT_full = target_spans.T.astype(np.float16)               # [H, N_TOT]
    aT_full = argument_spans.T.astype(np.float16)             # [H, M_TOT]
    # aT chunk-major: [128, MC, KT, MCH]
    aT_p = np.ascontiguousarray(
        aT_full.reshape(KT, 128, MC, MCH).transpose(1, 2, 0, 3))
    ut_p = perm_kp(U[:H].astype(np.float16))                  # [128, KT, 1]
    ua_p = U[H:].astype(np.float16).reshape(KT, 128).T        # [128, KT]

    tT_p = perm_kp(tT_full)                                   # [128,KT,N_TOT]
    in_maps = []
    for i in range(N_CORES):
        tT_aug = np.zeros((128, KT, NSH + 4), dtype=np.float16)
        tT_aug[:, :, 0:NSH] = tT_p[:, :, i * NSH:(i + 1) * NSH]
        tT_aug[:, :, NSH] = ut_p[:, :, 0]
        tT_aug[:, :, NSH + 1] = ut_p[:, :, 0]
        tT_aug[:, :, NSH + 2] = ua_p
        in_maps.append({
            "tT": tT_aug,
            "aT": aT_p,
            "W": W_p,
        })
    return in_maps, b_val


def kernel(target_spans, argument_spans, W, U, b):
    in_maps, b_val = make_in_maps(target_spans, argument_spans, W, U, b)
    nc = _get_nc(b_val)
    res = run_bass_kernel_spmd(nc, in_maps, core_ids=list(range(N_CORES)))
    out = np.concatenate(
        [res.results[i]["out"] for i in range(N_CORES)], axis=0
    )
    return out.astype(np.float32, copy=False)
